# revision 11
# baseline (speedup 1.0000x reference)
"""Trainium2 Bass kernel: per-pixel 5x5-patch channel covariance.

R[b,h,w,k,l] = (1/N) sum_n (p_kn - mu_k)(p_ln - mu_l)   (N=25, reflect pad)

Identity:  R = box5x5(S_k * S_l)/25 - mu_k * mu_l,  mu = box5x5(S)/25.
Separable box sums run as banded matmuls on TensorE; reflect padding is
folded into the band weights. Host pre-scales S by 1/5 so the two band
passes produce box/25 directly.

Optimizations over the naive version:
- Pair products in "diagonal" order: P_d = S[0:16-d] * S[d:16] -- plain
  slice-vs-slice tensor ops (no broadcast) so DVE runs in its 2x mode;
  the d=0 diagonal (squares) runs on the Activation engine.
- Device emits only the 136 unique (upper-tri) covariance channels in
  d-major order; the symmetric mirror happens on the host in assemble.
- PSUM traffic fused into [128,1024] ops and split between Activation
  and Vector; SBUF-only elementwise (products, mu-products) split
  between Vector and GpSimd (GPSIMD cannot touch PSUM).
- One DMA per (w-half, 16-channel block) with 4KB contiguous runs.

Sharding: 8 cores = 4 batches x 2 H-halves. Fully data parallel.
"""
import sys

sys.path.insert(0, "/opt/trn_rl_repo")

from contextlib import ExitStack

import numpy as np

import concourse.bacc as bacc
import concourse.mybir as mybir
import concourse.tile as tile
from concourse import bass_utils

B, K, H, W = 4, 16, 256, 256
HH = 128           # output rows per core
SR = 132           # shard rows (128 + 2 halo each side, reflect-mapped)
F32 = mybir.dt.float32
BF16 = mybir.dt.bfloat16

# channel order c' 0..151: 0..15 means; 16..151 pairs d-major:
#   d block at POFF[d], entries k=0..15-d -> pair (k, k+d)
POFF = [16]
for _d in range(1, 16):
    POFF.append(POFF[-1] + (17 - _d))          # prev block size; POFF[15]+1 == 152
NCH = 152
NG = NCH // 8                                  # 19 channel octets
NPAIR = 136

# pair channel list in device order (c' - 16)
PAIRS_DMAJOR = [(k, k + d) for d in range(16) for k in range(16 - d)]

# --- engine split knobs (tuned against TimelineSim) ---
import os as _os


def _knob(name, default):
    v = _os.environ.get(name)
    return default if v is None else eval(v)


PROD_POOL_DS = _knob("KN_PROD_POOL", (10, 11, 12, 13, 14, 15))
M_POOL_DS = _knob("KN_M_POOL", (9, 10, 11, 12, 13, 14, 15))
# I1 evacuation ops (38): "act" scalar-copy / "dve" vector-copy
EVAC_PATTERN = _knob("KN_EVAC", ("act", "act", "dve"))
# final subs (36): dve = DVE reads PSUM directly; act2 = Act evac + DVE
# SBUF sub; act2pool = Act evac + Pool SBUF sub
SUB_PATTERN = _knob("KN_SUB", ("act2", "dve", "act2", "act2pool", "act2", "dve"))
# squares engine: "act" or "dve"
SQ_ENG = _knob("KN_SQ", "act")


def _reflect_idx(i, n):
    if i < 0:
        return -i
    if i >= n:
        return 2 * (n - 1) - i
    return i


def _build_bw():
    """[128 w'col, 512]: blocks (oh, chunk): BW[:, (oh*2+c)*128 + wl] =
    M[c*128 + :, oh*128 + wl] where M[w', w_out] are box weights with
    reflection folded."""
    M = np.zeros((W, W), dtype=np.float32)
    for w in range(W):
        for j in range(5):
            M[_reflect_idx(w - 2 + j, W), w] += 1.0
    out = np.zeros((128, 512), dtype=np.float32)
    for oh in range(2):
        for c in range(2):
            out[:, (oh * 2 + c) * 128:(oh * 2 + c) * 128 + 128] = \
                M[c * 128:(c + 1) * 128, oh * 128:(oh + 1) * 128]
    return out


def _build_br(half):
    """[68, 128]: cols rt*64+hl; rows are shard-local rows within row-tile rt."""
    hbase = half * HH
    M = np.zeros((68, 128), dtype=np.float32)
    for rt in range(2):
        for hl in range(64):
            hg = hbase + rt * 64 + hl
            for i in range(5):
                r = _reflect_idx(hg - 2 + i, H)
                j = r + 2 - hbase          # canonical shard row
                M[j - rt * 64, rt * 64 + hl] += 1.0
    return M


def _build_kernel():
    nc = bacc.Bacc("TRN2", target_bir_lowering=False, debug=False)
    S_d = nc.dram_tensor("S", [SR, K, W], BF16, kind="ExternalInput").ap()
    BR_d = nc.dram_tensor("BR", [68, 128], BF16, kind="ExternalInput").ap()
    BW_d = nc.dram_tensor("BW", [128, 512], BF16, kind="ExternalInput").ap()
    # out: [oh, w 128, c 136, h 128] pair channels only (d-major)
    R_d = nc.dram_tensor("R", [2, 128, NPAIR, 128], BF16,
                         kind="ExternalOutput").ap()

    with tile.TileContext(nc) as tc, ExitStack() as ctx:
        const_p = ctx.enter_context(tc.tile_pool(name="const", bufs=1))
        sp_p = ctx.enter_context(tc.tile_pool(name="sp", bufs=1))
        t_p = ctx.enter_context(tc.tile_pool(name="tprod", bufs=6))
        sq_p = ctx.enter_context(tc.tile_pool(name="sq", bufs=2))
        i1_p = ctx.enter_context(tc.tile_pool(name="i1", bufs=4))
        mu_p = ctx.enter_context(tc.tile_pool(name="mu", bufs=2))
        m_p = ctx.enter_context(tc.tile_pool(name="mm", bufs=2))
        r_p = ctx.enter_context(tc.tile_pool(name="rout", bufs=3))
        sx_p = ctx.enter_context(tc.tile_pool(name="sx", bufs=2))
        ps1_p = ctx.enter_context(tc.tile_pool(name="ps1", bufs=2, space="PSUM"))
        ps2_p = ctx.enter_context(tc.tile_pool(name="ps2", bufs=2, space="PSUM"))

        br = const_p.tile([68, 128], BF16)
        bw = const_p.tile([128, 512], BF16)
        nc.sync.dma_start(br[:], BR_d)
        nc.sync.dma_start(bw[:], BW_d)

        sp0 = sp_p.tile([68, K, W], BF16)
        sp1 = sp_p.tile([68, K, W], BF16)
        nc.sync.dma_start(sp0[:], S_d[0:68])
        nc.sync.dma_start(sp1[:], S_d[64:132])
        sps = [sp0, sp1]
        brts = [br[:, 0:64], br[:, 64:128]]
        bwas = [bw[:, 0:128], bw[:, 256:384]]
        bwbs = [bw[:, 128:256], bw[:, 384:512]]

        state = {"evac": 0, "sub": 0}

        def pass1_g(rt, g, chunk_src):
            """H-box for channel octet g of row tile rt -> [128, 1024] bf16
            SBUF slice (layout [half][c8][h64]); returns the slice."""
            ps1 = ps1_p.tile([128, 1024], F32, name="ps1")
            for j in range(8):
                src, lc = chunk_src[g * 8 + j]
                for half in range(2):
                    nc.tensor.matmul(
                        ps1[:, half * 512 + j * 64:half * 512 + j * 64 + 64],
                        src[:, lc, half * 128:(half + 1) * 128],
                        brts[rt], start=True, stop=True)
            i1g = i1_p.tile([128, 1024], BF16, name="i1g")
            if EVAC_PATTERN[state["evac"] % len(EVAC_PATTERN)] == "act":
                nc.scalar.copy(i1g[:], ps1[:])
            else:
                nc.vector.tensor_copy(i1g[:], ps1[:])
            state["evac"] += 1
            return i1g

        def mm_pair(ps, oh, i1g, off):
            nc.tensor.matmul(ps[:, off:off + 512], bwas[oh],
                             i1g[:, 0:512], start=True, stop=False)
            nc.tensor.matmul(ps[:, off:off + 512], bwbs[oh],
                             i1g[:, 512:1024], start=False, stop=True)

        # ---- phase B helper (defined first; used for pre-build too) ----
        csrc = [{k: (sps[rt], k) for k in range(K)} for rt in range(2)]

        def build_products_upto(rt, cmax):
            src = csrc[rt]
            if 16 not in src:
                sq = sq_p.tile([68, 16, 256], BF16, name="sq")
                if SQ_ENG == "act":
                    nc.scalar.square(sq[:], sps[rt][:])
                else:
                    nc.vector.tensor_mul(sq[:], sps[rt][:], sps[rt][:])
                for k in range(K):
                    src[16 + k] = (sq, k)
            d = max((dd for dd in range(1, 16) if POFF[dd] in src), default=0)
            d += 1
            while d < 16 and POFF[d] <= cmax:
                n = 16 - d
                td = t_p.tile([68, 16, 256], BF16, name="td")
                eng = nc.gpsimd if d in PROD_POOL_DS else nc.vector
                eng.tensor_mul(td[:, 0:n, :], sps[rt][:, 0:n, :],
                               sps[rt][:, d:16, :])
                for k in range(n):
                    src[POFF[d] + k] = (td, k)
                d += 1

        # ---- issue order: squares + early products first so the DVE/Act
        # queues are not head-of-line blocked by the means dependency chain;
        # then the means phases (PE/Act); then M; then the pair pipeline.
        for rt in range(2):
            build_products_upto(rt, 5 * 8 + 7)

        # ---- phase A: means (g = 0,1) through both passes -> mub, M ----
        i1_mean = [[pass1_g(rt, g, csrc[rt]) for g in range(2)]
                   for rt in range(2)]
        mubs, Ms = [], []
        for oh in range(2):
            mub = mu_p.tile([128, K, 128], BF16, name="mub")
            for rt in range(2):
                ps2 = ps2_p.tile([128, 1024], F32, name="ps2")
                mm_pair(ps2, oh, i1_mean[rt][0], 0)
                mm_pair(ps2, oh, i1_mean[rt][1], 512)
                # mean channels carry one factor of the 0.2 prescale, not
                # two: ps2 here is 5*mu, so scale by 0.2 on evacuation.
                nc.scalar.mul(
                    mub[:, :, rt * 64:(rt + 1) * 64],
                    ps2[:].rearrange("p (c h) -> p c h", c=16), 0.2)
            mubs.append(mub)
        for oh in range(2):
            M = m_p.tile([128, NPAIR, 128], BF16, name="M")
            for d in range(16):
                n = 16 - d
                eng = nc.gpsimd if d in M_POOL_DS else nc.vector
                eng.tensor_mul(M[:, POFF[d] - 16:POFF[d] - 16 + n, :],
                               mubs[oh][:, 0:n, :], mubs[oh][:, d:16, :])
            Ms.append(M)

        pend = {}                                    # g -> [i1g(rt0), i1g(rt1)]
        for g in range(2, NG):
            pend[g] = []
            for rt in range(2):
                build_products_upto(rt, g * 8 + 7)
                pend[g].append(pass1_g(rt, g, csrc[rt]))
            if g % 2 == 0 and g != NG - 1:
                continue
            # emit pass-2 + sub + DMA for the completed group pair
            gp = g - 1 if g % 2 == 1 else g
            ng = 2 if g % 2 == 1 else 1
            nch = 8 * ng
            for oh in range(2):
                rtile = r_p.tile([128, 16, 128], BF16, name="rtile")
                for rt in range(2):
                    ps2 = ps2_p.tile([128, 1024], F32, name="ps2")
                    for gg in range(ng):
                        mm_pair(ps2, oh, pend[gp + gg][rt], gg * 512)
                    p2v = ps2[:, 0:nch * 64].rearrange(
                        "p (c h) -> p c h", c=nch)
                    ms = Ms[oh][:, (gp - 2) * 8:(gp - 2) * 8 + nch,
                                rt * 64:(rt + 1) * 64]
                    dstv = rtile[:, 0:nch, rt * 64:(rt + 1) * 64]
                    mode = SUB_PATTERN[state["sub"] % len(SUB_PATTERN)]
                    state["sub"] += 1
                    if mode == "act2":
                        sx = sx_p.tile([128, 16, 64], BF16, name="sx")
                        nc.scalar.copy(sx[:, 0:nch, :], p2v)
                        nc.vector.tensor_sub(dstv, sx[:, 0:nch, :], ms)
                    elif mode == "act2pool":
                        sx = sx_p.tile([128, 16, 64], BF16, name="sx")
                        nc.scalar.copy(sx[:, 0:nch, :], p2v)
                        nc.gpsimd.tensor_sub(dstv, sx[:, 0:nch, :], ms)
                    else:
                        nc.vector.tensor_sub(dstv, p2v, ms)
                nc.sync.dma_start(
                    R_d[oh][:, (gp - 2) * 8:(gp - 2) * 8 + nch, :],
                    rtile[:, 0:nch, :])

    nc.compile()
    return nc


_NC_CACHE = {}


def _get_nc():
    if "nc" not in _NC_CACHE:
        _NC_CACHE["nc"] = _build_kernel()
    return _NC_CACHE["nc"]


def _prep_in_maps(S):
    S = np.asarray(S, dtype=np.float32)
    np_bf16 = mybir.dt.np(BF16)
    bw = _build_bw().astype(np_bf16)
    brs = [(_build_br(h)).astype(np_bf16) for h in range(2)]
    Ss = S * np.float32(0.2)
    in_maps = []
    for b in range(B):
        for half in range(2):
            hbase = half * HH
            rows = np.clip(np.arange(hbase - 2, hbase + 130), 0, H - 1)
            shard = Ss[b][:, rows, :].transpose(1, 0, 2)   # [132, K, 256]
            shard = np.ascontiguousarray(shard).astype(np_bf16)
            in_maps.append({"S": shard, "BR": brs[half], "BW": bw})
    return in_maps


# host-side scatter index: device pair slot -> (k, l)
_IDX_K = np.array([p[0] for p in PAIRS_DMAJOR])
_IDX_L = np.array([p[1] for p in PAIRS_DMAJOR])


def _assemble(results):
    out = np.empty((B, H, W, K, K), dtype=np.float32)
    for i in range(8):
        b, half = divmod(i, 2)
        r = np.asarray(results[i]["R"]).astype(np.float32)
        # r: [oh, w 128, c 136, h 128] -> [h, w, pair 136]
        r = r.reshape(2, 128, NPAIR, 128)
        r = r.transpose(3, 0, 1, 2)                # h, oh, w, c
        r = r.reshape(HH, W, NPAIR)
        blk = out[b, half * HH:(half + 1) * HH]
        blk[:, :, _IDX_K, _IDX_L] = r
        blk[:, :, _IDX_L, _IDX_K] = r
    return out


def kernel(S):
    """S: [4, 16, 256, 256] float32 -> R: [4, 256, 256, 16, 16] float32."""
    nc = _get_nc()
    in_maps = _prep_in_maps(S)
    res = bass_utils.run_bass_kernel_spmd(nc, in_maps, list(range(8)))
    return _assemble(res.results)
